# revision 1
# baseline (speedup 1.0000x reference)
"""CMRET equivariant message-passing GNN — Trainium2 Bass kernel.

Structure exploited: the batch mask is block-diagonal (8 molecules x 32
contiguous atoms) and every pairwise term (cutoff, RBF, attention mask) is
zero across molecules, so the 8 molecules are fully independent through the
whole network. We run one molecule per NeuronCore (8 cores), dense 32x32
local attention per molecule, and no collectives.

Per-core layout: feature-on-partition (128 feature partitions, free axis =
1024 edges (a*32+b) or 32 atoms). PE does all head-broadcast / head-sum /
partition-stat contractions with constant 0/1 matrices; ACT does
silu/exp/ln/sin (table sets: exp-set for geometry+layers, silu-set for the
static edge-MLP phase); DVE does the per-edge products and segmented
reductions.
"""

import numpy as np

RC = 5.0
N_ATOM = 256
N_MOL = 8
NA = 32          # atoms per molecule
F = 128
K = 50
L = 4
H = 4
Dh = 32
TEMP = 2.0
NE = NA * NA     # dense per-molecule edges (diag masked)
GAMMA = 0.5 / (RC / (K - 1)) ** 2
TEMPERATURE = TEMP * np.sqrt(Dh)
PI = float(np.pi)


def _wall_layout():
    """Packed constant layout: list of (name, partitions, cols)."""
    ent = []
    for l in range(L):
        ent += [(f"Wq{l}", F, F), (f"bq{l}", F, 1), (f"Wk{l}", F, F), (f"bk{l}", F, 1),
                (f"Wv{l}", F, 3 * F), (f"bv{l}", F, 3), (f"Wdk{l}", K, F), (f"bdk{l}", F, 1),
                (f"Wdv{l}", K, 3 * F), (f"bdv{l}", F, 3), (f"Wo{l}", F, 3 * F), (f"bo{l}", F, 3),
                (f"U1{l}", F, F), (f"U2{l}", F, F), (f"U3{l}", F, F)]
    ent += [("w1p", F, F // 2), ("b1p", F // 2, 1), ("w2", F // 2, 1), ("HH", F, F),
            ("halfdmask", NA, NA), ("diagI", NA, NA), ("mub", K, 1),
            ("ones128inv", F, 1), ("ones1", 1, F), ("I128", F, F),
            ("s0T", F, NA), ("R", NA, 3)]
    offs = {}
    c = 0
    for n, p, w in ent:
        offs[n] = (c, p, w)
        c += w
    # bf16 wall (PE fast-path operands)
    enth = [(f"Wdk{l}", K, F) for l in range(L)] + \
           [(f"Wdv{l}", K, 3 * F) for l in range(L)] + [("HH", F, F)]
    offsh = {}
    ch = 0
    for n, p, w in enth:
        offsh[n] = (ch, p, w)
        ch += w
    return offs, c, offsh, ch


def _host_prep(inp):
    """Fold LN affine + temperature into weights; pack into one Wall tensor
    per core; shard per molecule."""
    f32 = np.float32
    Z = np.asarray(inp["Z"]).reshape(-1)            # (256,)
    Rfull = np.asarray(inp["R"], f32).reshape(N_ATOM, 3)
    embed = np.asarray(inp["embed"], f32)
    s0 = embed[Z]                                   # (256, F) gather on host

    vals = {}
    for l in range(L):
        g = np.asarray(inp["ln_g"][l], f32)
        b = np.asarray(inp["ln_b"][l], f32)
        Wq = np.asarray(inp["Wq"][l], f32)
        Wk = np.asarray(inp["Wk"][l], f32)
        Wv = np.asarray(inp["Wv"][l], f32)
        vals[f"Wq{l}"] = g[:, None] * Wq / TEMPERATURE
        vals[f"bq{l}"] = (b @ Wq / TEMPERATURE).reshape(F, 1)
        vals[f"Wk{l}"] = g[:, None] * Wk
        vals[f"bk{l}"] = (b @ Wk).reshape(F, 1)
        vals[f"Wv{l}"] = g[:, None] * Wv
        vals[f"bv{l}"] = (b @ Wv).reshape(3, F).T
        vals[f"Wdk{l}"] = np.asarray(inp["Wdk"][l], f32)
        vals[f"bdk{l}"] = np.asarray(inp["bdk"][l], f32).reshape(F, 1)
        vals[f"Wdv{l}"] = np.asarray(inp["Wdv"][l], f32)
        vals[f"bdv{l}"] = np.asarray(inp["bdv"][l], f32).reshape(3, F).T
        vals[f"Wo{l}"] = np.asarray(inp["Wo"][l], f32)
        vals[f"bo{l}"] = np.asarray(inp["bo"][l], f32).reshape(3, F).T
        vals[f"U1{l}"] = np.asarray(inp["U1"][l], f32)
        vals[f"U2{l}"] = np.asarray(inp["U2"][l], f32)
        vals[f"U3{l}"] = np.asarray(inp["U3"][l], f32)

    lg = np.asarray(inp["lnf_g"], f32)
    lb = np.asarray(inp["lnf_b"], f32)
    w1 = np.asarray(inp["out_w1"], f32)
    vals["w1p"] = lg[:, None] * w1
    vals["b1p"] = (lb @ w1 + np.asarray(inp["out_b1"], f32)).reshape(F // 2, 1)
    vals["w2"] = np.asarray(inp["out_w2"], f32).reshape(F // 2, 1)

    hh = np.zeros((F, F), f32)
    for h in range(H):
        hh[h * Dh:(h + 1) * Dh, h * Dh:(h + 1) * Dh] = 1.0
    vals["HH"] = hh
    eye = np.eye(NA, dtype=f32)
    vals["halfdmask"] = (0.5 * (1.0 - eye)).astype(f32)
    vals["diagI"] = eye
    mu = np.linspace(0.0, RC, K).astype(f32)
    vals["mub"] = (-np.sqrt(GAMMA) * mu).reshape(K, 1).astype(f32)
    vals["ones128inv"] = np.full((F, 1), 1.0 / F, f32)
    vals["ones1"] = np.ones((1, F), f32)
    vals["I128"] = np.eye(F, dtype=f32)

    offs, C, offsh, CH = _wall_layout()
    base = np.zeros((F, C), f32)
    for n, v in vals.items():
        c0, p, w = offs[n]
        base[0:p, c0:c0 + w] = v
    import ml_dtypes
    wallh = np.zeros((F, CH), dtype=ml_dtypes.bfloat16)
    for n, (c0, p, w) in offsh.items():
        wallh[0:p, c0:c0 + w] = vals[n].astype(ml_dtypes.bfloat16)
    wallh = np.ascontiguousarray(wallh)
    walls = []
    for m in range(N_MOL):
        wl = base.copy()
        c0, p, w = offs["s0T"]
        wl[0:p, c0:c0 + w] = s0[m * NA:(m + 1) * NA].T
        c0, p, w = offs["R"]
        wl[0:p, c0:c0 + w] = Rfull[m * NA:(m + 1) * NA]
        walls.append(np.ascontiguousarray(wl))
    b2 = float(np.asarray(inp["out_b2"]).reshape(-1)[0])
    return walls, wallh, b2


_CACHE = {}


def kernel(**inputs):
    from concourse import bass_utils

    walls, wallh, b2 = _host_prep(inputs)

    key = ("nc", b2)
    if key not in _CACHE:
        _CACHE[key] = _build(b2)
    nc = _CACHE[key]

    in_maps = [{"Wall": walls[m], "WallH": wallh} for m in range(N_MOL)]
    res = bass_utils.run_bass_kernel_spmd(nc, in_maps, core_ids=list(range(N_MOL)))
    out = np.concatenate([r["energy"].reshape(1) for r in res.results]).reshape(N_MOL, 1)
    return out.astype(np.float32)


def _patch_tile_drain():
    """The Tile kernel-tail drain carries one sem-wait per active processor;
    this walrus build caps sync waits per CTRL instruction. Split the waits
    onto individual SP nops (same semantics: all run before the exit
    barrier on the sync engine)."""
    import concourse.tile as tile_mod
    import bass_rust
    from concourse.vector_clock import ScopedClock

    if getattr(tile_mod.TileContext, "_drain_split_patched", False):
        return

    def _drain_and_barrier(self, tick_clock, wait_clock):
        nc = self.nc
        drain_inst = nc.sync.drain()
        wait_clock.add_sem_waits(
            drain_inst.ins, ScopedClock({None: tick_clock.global_clock})
        )
        si = drain_inst.ins.sync_info
        waits = list(si.on_wait or []) if si is not None else []
        if len(waits) > 1:
            drain_inst.ins.sync_info = bass_rust.SyncInfo(
                on_wait=waits[:1], on_update=list(si.on_update or []))
            for w in waits[1:]:
                nop = nc.sync.nop(nofuse=True)
                nop.ins.sync_info = bass_rust.SyncInfo(on_wait=[w], on_update=[])
        nc.all_engine_barrier()
        popped = nc._tile_sem_poison_stack.pop()
        assert popped is self._sem_poison
        nc.clear_and_free_semaphores(list(self.sems.allocated().values()))
        nc.all_engine_barrier()

    tile_mod.TileContext._drain_and_barrier = _drain_and_barrier
    tile_mod.TileContext._drain_split_patched = True


def _split_sync_waits(nc, mybir):
    """This walrus build rejects instructions carrying more than one sync
    wait ("Too many sync wait commands"). Hoist extra waits onto inserted
    same-engine NoOps immediately before the instruction — the engine
    sequencer blocks on each in turn, preserving the happens-before."""
    import bass_rust

    n_split = 0
    for fn in nc.m.functions:
        for bb in fn.blocks:
            changed = False
            new = []
            for ins in bb.instructions:
                si = ins.sync_info
                waits = list(si.on_wait or []) if si is not None else []
                if len(waits) > 1:
                    for i, w in enumerate(waits[:-1]):
                        nop = mybir.InstNoOp(name=f"{ins.name}-sw{i}")
                        nop.engine = ins.engine
                        nop.sync_info = bass_rust.SyncInfo(on_wait=[w], on_update=[])
                        nc.inst_map[nop.name] = nop
                        new.append(nop)
                    ins.sync_info = bass_rust.SyncInfo(
                        on_wait=[waits[-1]], on_update=list(si.on_update or []))
                    changed = True
                    n_split += 1
                new.append(ins)
            if changed:
                bb.instructions = new
    return n_split


def _build(b2, silu_native=True):
    # silu_native=False replaces the native Silu ACT op (not implemented by
    # CoreSim) with z*sigmoid(z) for simulator-based testing only.
    import concourse.bass as bass
    import concourse.mybir as mybir
    import concourse.tile as tile

    _patch_tile_drain()

    f32 = mybir.dt.float32
    AF = mybir.ActivationFunctionType
    ALU = mybir.AluOpType
    AX = mybir.AxisListType

    def bcast_inner(ap, outer, inner):
        # (P, n) -> (P, outer(step), inner(bcast)): value[p, i, j] = ap[p, i]
        return bass.AP(tensor=ap.tensor, offset=ap.offset,
                       ap=[ap.ap[0], [ap.ap[1][0], outer], [0, inner]])

    def bcast_outer(ap, outer, inner):
        # (P, n) -> (P, outer(bcast), inner(step)): value[p, i, j] = ap[p, j]
        return bass.AP(tensor=ap.tensor, offset=ap.offset,
                       ap=[ap.ap[0], [0, outer], [ap.ap[1][0], inner]])

    bf16 = mybir.dt.bfloat16
    fp16 = mybir.dt.float16
    nc = bass.Bass()
    offs, C, offsh, CH = _wall_layout()
    Wall = nc.dram_tensor("Wall", [F, C], f32, kind="ExternalInput")
    WallH = nc.dram_tensor("WallH", [F, CH], bf16, kind="ExternalInput")
    energy = nc.dram_tensor("energy", [1, 1], f32, kind="ExternalOutput")

    with tile.TileContext(nc) as tc:
        with tc.tile_pool(name="const", bufs=1) as cp, \
             tc.tile_pool(name="geo", bufs=1) as gp, \
             tc.tile_pool(name="small", bufs=2) as sp, \
             tc.tile_pool(name="wide", bufs=7) as wp, \
             tc.tile_pool(name="psW", bufs=1, space="PSUM") as psW, \
             tc.tile_pool(name="psS", bufs=3, space="PSUM") as psS:

            # ---- load all constants/weights with ONE DMA ----
            wall = cp.tile([F, C], f32, tag="wall", name="wall")
            nc.sync.dma_start(out=wall[:], in_=Wall[:])
            W = {}
            for n, (c0, p, w) in offs.items():
                W[n] = wall[0:p, c0:c0 + w]
            wallht = cp.tile([F, CH], bf16, tag="wallh", name="wallht")
            nc.sync.dma_start(out=wallht[:], in_=WallH[:])
            WH = {}
            for n, (c0, p, w) in offsh.items():
                WH[n] = wallht[0:p, c0:c0 + w]
            # fp16 identity + bf16 ones for cheap non-fp32 PE ops
            I128h = cp.tile([F, F], fp16, tag="I128h", name="I128h")
            nc.vector.tensor_copy(I128h[:], W["I128"])
            ones1h = cp.tile([1, F], bf16, tag="ones1h", name="ones1h")
            nc.vector.tensor_copy(ones1h[:], W["ones1"])

            # small constant bias tiles for ACT (only 0.0/1.0 have const APs)
            b30 = cp.tile([NA, 1], f32, tag="b30", name="b30")
            nc.vector.memset(b30[:], 1e-30)
            bpi2 = cp.tile([NA, 1], f32, tag="bpi2", name="bpi2")
            nc.vector.memset(bpi2[:], PI / 2)
            beps = cp.tile([1, 1], f32, tag="beps", name="beps")
            nc.vector.memset(beps[:], 1e-5)

            # =========== geometry (ACT: exp/ln set) ===========
            Rb = gp.tile([NA, NA * 3], f32, tag="Rb")   # R[b, c] replicated over a
            rc0 = offs["R"][0]
            wap = Wall[:]
            nc.sync.dma_start(out=Rb[:], in_=bass.AP(tensor=wap.tensor, offset=rc0,
                                                     ap=[[0, NA], [C, NA], [1, 3]]))
            V = gp.tile([NA, NA, 3], f32, tag="V")      # vec[a, b, c] = R[a,c] - R[b,c]
            Ra = W["R"][:]
            Ra_b = bass.AP(tensor=Ra.tensor, offset=Ra.offset,
                           ap=[Ra.ap[0], [0, NA], [Ra.ap[1][0], 3]])
            nc.vector.tensor_sub(V[:], Ra_b, Rb[:].rearrange("p (b c) -> p b c", c=3))
            V2 = sp.tile([NA, NA, 3], f32, tag="V2")
            nc.vector.tensor_mul(V2[:], V[:], V[:])
            d2 = sp.tile([NA, NA], f32, tag="d2")
            nc.vector.reduce_sum(d2[:], V2[:], axis=AX.X)
            lnd2 = sp.tile([NA, NA], f32, tag="lnd2")
            nc.scalar.activation(lnd2[:], d2[:], AF.Ln, bias=b30[:])
            dmat = gp.tile([NA, NA], f32, tag="dmat")   # d = exp(0.5*ln(d2))
            nc.scalar.activation(dmat[:], lnd2[:], AF.Exp, scale=0.5)
            dsafe = sp.tile([NA, NA], f32, tag="dsafe")
            nc.vector.tensor_add(dsafe[:], dmat[:], W["diagI"][:])
            invd = sp.tile([NA, NA], f32, tag="invd")
            nc.vector.reciprocal(invd[:], dsafe[:])
            vn = gp.tile([NA, NA, 3], f32, tag="vn")    # vec_norm (diag exactly 0)
            iap = invd[:]
            nc.vector.tensor_mul(vn[:], V[:], bass.AP(tensor=iap.tensor, offset=iap.offset,
                                                      ap=[iap.ap[0], [iap.ap[1][0], NA], [0, 3]]))
            vn_b = gp.tile([NA, NA, 3], bf16, tag="vn_b", name="vn_b")
            nc.vector.tensor_copy(vn_b[:], vn[:])
            vnrow = [gp.tile([1, NE], bf16, tag=f"vnrow{c}", name=f"vnrow{c}")
                     for c in range(3)]
            for c in range(3):
                nc.sync.dma_start(out=vnrow[c][:], in_=vn_b[:, :, c])

            # RBF edge features eT (K, NE): broadcast d across partitions with
            # a K=1 ones-matmul; ACT reads the PSUM directly
            dE = gp.tile([1, NE], f32, tag="dE", name="dE")
            nc.sync.dma_start(out=dE[:], in_=dmat[:])
            erbf = gp.tile([K, NE], f32, tag="erbf", name="erbf")
            for hh_ in range(2):
                sl = slice(hh_ * 512, (hh_ + 1) * 512)
                pb = psW.tile([K, 512], f32, tag="bcast", name="pb_rbf")
                nc.tensor.matmul(pb[:], W["ones1"][0:1, 0:K], dE[:, sl],
                                 start=True, stop=True)
                nc.scalar.activation(erbf[:, sl], pb[:], AF.Square, bias=W["mub"][:],
                                     scale=float(np.sqrt(GAMMA)))
            nc.scalar.activation(erbf[:], erbf[:], AF.Exp, scale=-1.0)
            e1 = erbf

            # fence: make sin's input depend on e1 so the ACT engine finishes
            # all exp-set work before the silu/sin set loads
            d_g = sp.tile([NA, NA], f32, tag="d_g")
            nc.vector.scalar_tensor_tensor(d_g[:], e1[0:NA, 0:NA], 0.0, dmat[:],
                                           op0=ALU.mult, op1=ALU.add)

            # =========== cutoff + edge MLPs (ACT: silu set) ===========
            stepm = sp.tile([NA, NA], f32, tag="stepm")
            nc.vector.tensor_scalar(out=stepm[:], in0=d_g[:], scalar1=RC, scalar2=None,
                                    op0=ALU.is_le)
            # clamp d to RC so the sin argument stays in [-pi/2, pi/2];
            # cos(pi*d/RC) = sin(pi/2 - pi*d/RC)
            nc.vector.tensor_scalar(out=d_g[:], in0=d_g[:], scalar1=RC, scalar2=None,
                                    op0=ALU.min)
            s1 = sp.tile([NA, NA], f32, tag="s1")
            nc.scalar.activation(s1[:], d_g[:], AF.Sin, bias=bpi2[:], scale=-PI / RC)
            m32 = sp.tile([NA, NA], f32, tag="m32")
            nc.vector.tensor_mul(m32[:], stepm[:], W["halfdmask"][:])
            co_a = gp.tile([NA, NA], f32, tag="co_a")   # cosine cutoff * pair mask
            nc.vector.scalar_tensor_tensor(co_a[:], s1[:], 1.0, m32[:],
                                           op0=ALU.add, op1=ALU.mult)
            co_b = gp.tile([NA, NA], bf16, tag="co_b", name="co_b")
            nc.vector.tensor_copy(co_b[:], co_a[:])
            coE = gp.tile([1, NE], bf16, tag="coE", name="coE")
            nc.sync.dma_start(out=coE[:], in_=co_b[:])
            co50 = gp.tile([K, NE], bf16, tag="co50")
            for hh_ in range(2):
                sl = slice(hh_ * 512, (hh_ + 1) * 512)
                pb = psW.tile([K, 512], f32, tag="bcast", name="pb_co50")
                nc.tensor.matmul(pb[:], ones1h[0:1, 0:K], coE[:, sl],
                                 start=True, stop=True)
                nc.scalar.copy(co50[:, sl], pb[:])
            vnE = []
            for c in range(3):
                t = gp.tile([F, NE], bf16, tag=f"vnE{c}", name=f"vnE{c}")
                for hh_ in range(2):
                    sl = slice(hh_ * 512, (hh_ + 1) * 512)
                    pb = psW.tile([F, 512], f32, tag="bcast", name="pb_vne")
                    nc.tensor.matmul(pb[:], ones1h[:], vnrow[c][:, sl],
                                     start=True, stop=True)
                    nc.scalar.copy(t[:, sl], pb[:])
                vnE.append(t)
            e_full = gp.tile([K, NE], bf16, tag="e_full", name="e_full")
            nc.vector.tensor_mul(e_full[:], e1[:], co50[:])

            # static edge MLPs for all layers: dk = silu(e@Wdk+bdk), dv_e = silu(e@Wdv+bdv)
            def edge_silu(dst, pm, bias_ap):
                if silu_native:
                    nc.scalar.activation(dst, pm[:], AF.Silu, bias=bias_ap)
                else:
                    sg = sp.tile([F, 512], f32, tag="sg", name="sg")
                    nc.scalar.activation(sg[:], pm[:], AF.Sigmoid, bias=bias_ap)
                    zz = sp.tile([F, 512], f32, tag="zz", name="zz")
                    nc.vector.tensor_scalar(out=zz[:], in0=pm[:], scalar1=bias_ap,
                                            scalar2=None, op0=ALU.add)
                    nc.vector.tensor_mul(dst, zz[:], sg[:])

            dkT, dvT = [], []
            for l in range(L):
                dk = gp.tile([F, NE], bf16, tag=f"dk{l}")
                for h in range(2):
                    pm = psW.tile([F, 512], f32, tag="mlp")
                    nc.tensor.matmul(pm[:], WH[f"Wdk{l}"], e_full[:, h * 512:(h + 1) * 512],
                                     start=True, stop=True)
                    edge_silu(dk[:, h * 512:(h + 1) * 512], pm, W[f"bdk{l}"][:])
                dkT.append(dk)
                dvl = []
                for c in range(3):
                    dv = gp.tile([F, NE], bf16, tag=f"dv{l}_{c}")
                    for h in range(2):
                        pm = psW.tile([F, 512], f32, tag="mlp")
                        nc.tensor.matmul(pm[:], WH[f"Wdv{l}"][:, c * F:(c + 1) * F],
                                         e_full[:, h * 512:(h + 1) * 512], start=True, stop=True)
                        edge_silu(dv[:, h * 512:(h + 1) * 512], pm, W[f"bdv{l}"][:, c:c + 1])
                    dvl.append(dv)
                dvT.append(dvl)

            # =========== back to exp/ln set for the layer loop ===========
            # fence: ln(co128)'s bias depends on the last silu tile
            tiny = sp.tile([F, 1], f32, tag="tiny")
            nc.vector.tensor_scalar(out=tiny[:], in0=dvT[L - 1][2][:, 0:1], scalar1=0.0,
                                    scalar2=1e-38, op0=ALU.mult, op1=ALU.add)
            lnco = gp.tile([F, NE], fp16, tag="lnco", name="lnco")
            for hh_ in range(2):
                sl = slice(hh_ * 512, (hh_ + 1) * 512)
                pb = psW.tile([F, 512], f32, tag="bcast", name="pb_lnco")
                nc.tensor.matmul(pb[:], ones1h[:], coE[:, sl], start=True, stop=True)
                nc.scalar.activation(lnco[:, sl], pb[:], AF.Ln, bias=tiny[:])

            # persistent state
            sT = gp.tile([F, NA], f32, tag="sT")
            nc.vector.tensor_copy(sT[:], W["s0T"][:])
            oT = gp.tile([F, NA], f32, tag="oT")
            nc.vector.memset(oT[:], 0.0)
            vT = []
            for c in range(3):
                t = gp.tile([F, NA], f32, tag=f"vT{c}")
                nc.vector.memset(t[:], 0.0)
                vT.append(t)

            def layernorm_f(inT, eps=1e-5):
                # LN stats over the feature (partition) axis via PE ones-matmuls
                sq = sp.tile([F, NA], f32, tag="lnsq")
                nc.scalar.activation(sq[:], inT[:], AF.Square)
                statm = psS.tile([1, NA], f32, tag="nmm")
                nc.tensor.matmul(statm[:], W["ones128inv"][:], inT[:], start=True, stop=True)
                stat2 = psS.tile([1, NA], f32, tag="nmm")
                nc.tensor.matmul(stat2[:], W["ones128inv"][:], sq[:], start=True, stop=True)
                musq = sp.tile([1, NA], f32, tag="musq")
                nc.scalar.activation(musq[:], statm[:], AF.Square)
                varr = sp.tile([1, NA], f32, tag="varr")
                nc.vector.scalar_tensor_tensor(varr[:], musq[:], -1.0, stat2[:],
                                               op0=ALU.mult, op1=ALU.add)
                lnv = sp.tile([1, NA], f32, tag="lnv")
                nc.scalar.activation(lnv[:], varr[:], AF.Ln, bias=beps[:])
                rb = sp.tile([1, 2 * NA], f32, tag="rb")
                nc.scalar.activation(rb[:, 0:NA], lnv[:], AF.Exp, scale=-0.5)   # rstd
                nc.vector.tensor_mul(rb[:, NA:2 * NA], statm[:], rb[:, 0:NA])  # mu*rstd
                bc = psS.tile([F, 2 * NA], f32, tag="nmm")
                nc.tensor.matmul(bc[:], W["ones1"][:], rb[:], start=True, stop=True)
                xm = sp.tile([F, NA], f32, tag="xm")
                nc.vector.tensor_mul(xm[:], inT[:], bc[:, 0:NA])
                xh = sp.tile([F, NA], f32, tag="xhatT")
                nc.vector.tensor_sub(xh[:], xm[:], bc[:, NA:2 * NA])
                return xh

            def e3(t):
                return t[:].rearrange("p (a b) -> p a b", a=NA)

            # =========== interaction layers ===========
            for l in range(L):
                first = l == 0
                xhatT = layernorm_f(sT)

                def node_mm(wname, bname, nch, tagp, dt=f32):
                    outs = []
                    for c in range(nch):
                        pm = psS.tile([F, NA], f32, tag="nmm")
                        wap = W[wname][:, c * F:(c + 1) * F] if nch > 1 else W[wname][:]
                        nc.tensor.matmul(pm[:], wap, xhatT[:], start=True, stop=True)
                        t = sp.tile([F, NA], dt, tag=f"{tagp}{c}")
                        bap = W[bname][:, c:c + 1] if nch > 1 else W[bname][:]
                        nc.scalar.activation(t[:], pm[:], AF.Identity, bias=bap)
                        outs.append(t)
                    return outs

                (qT,) = node_mm(f"Wq{l}", f"bq{l}", 1, "qT")
                (kT,) = node_mm(f"Wk{l}", f"bk{l}", 1, "kT")
                val = node_mm(f"Wv{l}", f"bv{l}", 3, "val", dt=bf16)

                # logits products: prod = (q ⊗ k) ⊙ dk   (in-place on qk)
                qk = wp.tile([F, NA, NA], f32, tag="w")
                prod_b = wp.tile([F, NA, NA], bf16, tag="w")
                nc.vector.tensor_mul(qk[:], bcast_inner(qT[:], NA, NA),
                                     bcast_outer(kT[:], NA, NA))
                nc.vector.tensor_mul(prod_b[:], qk[:], e3(dkT[l]))
                prod = prod_b[:].rearrange("p a b -> p (a b)")

                # head-summed logits via HH matmul; X = exp(L); Ec = exp(L + ln co)
                Xp = wp.tile([F, NE], f32, tag="w")
                Ec = wp.tile([F, NE], bf16, tag="w")
                for hch in range(2):
                    sl = slice(hch * 512, (hch + 1) * 512)
                    lfA = psW.tile([F, 512], f32, tag="lfA")
                    nc.tensor.matmul(lfA[:], WH["HH"], prod[:, sl], start=True, stop=True)
                    nc.scalar.activation(Xp[:, sl], lfA[:], AF.Exp)
                    lfB = psW.tile([F, 512], f32, tag="lfB")
                    nc.tensor.matmul(lfB[:], WH["HH"], prod[:, sl], start=True, stop=False)
                    nc.tensor.matmul(lfB[:], I128h[:], lnco[:, sl], start=False, stop=True)
                    nc.scalar.activation(Ec[:, sl], lfB[:], AF.Exp)

                # softmax denominator D = sum_b exp(l) - diag
                S = sp.tile([F, NA], f32, tag="S")
                nc.vector.reduce_sum(S[:], e3(Xp), axis=AX.X)
                xap = Xp[:]
                diag_ap = bass.AP(tensor=xap.tensor, offset=xap.offset,
                                  ap=[xap.ap[0], [(NA + 1) * xap.ap[1][0], NA]])
                invD = sp.tile([F, NA], f32, tag="invD")
                nc.vector.tensor_sub(invD[:], S[:], diag_ap)
                nc.vector.reciprocal(invD[:], invD[:])

                # ds message: sum_b Ec*dv1*val1[b]
                P1 = wp.tile([F, NA, NA], bf16, tag="w")
                nc.vector.tensor_mul(P1[:], e3(Ec), e3(dvT[l][0]))
                nc.vector.tensor_mul(P1[:], P1[:], bcast_outer(val[0][:], NA, NA))
                dsT = sp.tile([F, NA], f32, tag="dsT")
                nc.vector.reduce_sum(dsT[:], P1[:], axis=AX.X)
                nc.vector.tensor_mul(dsT[:], dsT[:], invD[:])

                # dw messages
                P3 = wp.tile([F, NA, NA], bf16, tag="w")
                nc.vector.tensor_mul(P3[:], e3(Ec), e3(dvT[l][2]))
                nc.vector.tensor_mul(P3[:], P3[:], bcast_outer(val[2][:], NA, NA))
                if not first:
                    P2 = wp.tile([F, NA, NA], bf16, tag="w")
                    nc.vector.tensor_mul(P2[:], e3(Ec), e3(dvT[l][1]))
                dwT = []
                for c in range(3):
                    if first:
                        tt = wp.tile([F, NA, NA], bf16, tag="w")
                        nc.vector.tensor_mul(tt[:], P3[:], e3(vnE[c]))
                    else:
                        G = sp.tile([F, NA], bf16, tag=f"G{c}")
                        nc.vector.tensor_mul(G[:], val[1][:], vT[c][:])
                        tt = wp.tile([F, NA, NA], bf16, tag="w")
                        nc.vector.tensor_mul(tt[:], P2[:], bcast_outer(G[:], NA, NA))
                        rr = wp.tile([F, NA, NA], bf16, tag="w")
                        nc.vector.tensor_mul(rr[:], P3[:], e3(vnE[c]))
                        nc.vector.tensor_add(tt[:], tt[:], rr[:])
                    dw = sp.tile([F, NA], f32, tag=f"dw{c}")
                    nc.vector.reduce_sum(dw[:], tt[:], axis=AX.X)
                    nc.vector.tensor_mul(dw[:], dw[:], invD[:])
                    dwT.append(dw)

                # gated node update
                oTs = []
                for c in range(3):
                    pm = psS.tile([F, NA], f32, tag="nmm")
                    nc.tensor.matmul(pm[:], W[f"Wo{l}"][:, c * F:(c + 1) * F], dsT[:],
                                     start=True, stop=True)
                    t = sp.tile([F, NA], f32, tag=f"oo{c}")
                    nc.scalar.activation(t[:], pm[:], AF.Identity,
                                         bias=W[f"bo{l}"][:, c:c + 1])
                    oTs.append(t)
                if first:
                    dx = oTs[1]
                    for c in range(3):
                        nc.vector.tensor_copy(vT[c][:], dwT[c][:])
                else:
                    dot = sp.tile([F, NA], f32, tag="dot")
                    vec3s = []
                    for c in range(3):
                        p1 = psS.tile([F, NA], f32, tag="nmm")
                        nc.tensor.matmul(p1[:], W[f"U1{l}"][:], vT[c][:], start=True, stop=True)
                        v1s = sp.tile([F, NA], f32, tag="v1s")
                        nc.scalar.copy(v1s[:], p1[:])
                        p2 = psS.tile([F, NA], f32, tag="nmm")
                        nc.tensor.matmul(p2[:], W[f"U2{l}"][:], vT[c][:], start=True, stop=True)
                        pc = sp.tile([F, NA], f32, tag="dotp")
                        nc.vector.tensor_mul(pc[:], v1s[:], p2[:])
                        if c == 0:
                            nc.vector.tensor_copy(dot[:], pc[:])
                        else:
                            nc.vector.tensor_add(dot[:], dot[:], pc[:])
                        p3 = psS.tile([F, NA], f32, tag="nmm")
                        nc.tensor.matmul(p3[:], W[f"U3{l}"][:], vT[c][:], start=True, stop=True)
                        v3s = sp.tile([F, NA], f32, tag=f"v3s{c}")
                        nc.scalar.copy(v3s[:], p3[:])
                        vec3s.append(v3s)
                    dx = sp.tile([F, NA], f32, tag="dx")
                    nc.vector.tensor_mul(dx[:], oTs[2][:], dot[:])
                    nc.vector.tensor_add(dx[:], dx[:], oTs[1][:])
                    for c in range(3):
                        t3 = sp.tile([F, NA], f32, tag="t3")
                        nc.vector.tensor_mul(t3[:], oTs[0][:], vec3s[c][:])
                        nc.vector.tensor_add(vT[c][:], vT[c][:], dwT[c][:])
                        nc.vector.tensor_add(vT[c][:], vT[c][:], t3[:])
                nc.vector.tensor_add(sT[:], sT[:], dx[:])
                nc.vector.tensor_add(oT[:], oT[:], dx[:])

            # =========== final LN + output MLP ===========
            xo = layernorm_f(oT)
            y_p = psS.tile([F // 2, NA], f32, tag="nmm")
            nc.tensor.matmul(y_p[:], W["w1p"][:], xo[:], start=True, stop=True)
            y = sp.tile([F // 2, NA], f32, tag="y")
            nc.scalar.activation(y[:], y_p[:], AF.Identity, bias=W["b1p"][:])
            # silu(y) = y / (1 + exp(-y)) using the resident exp table set
            ey = sp.tile([F // 2, NA], f32, tag="ey")
            nc.scalar.activation(ey[:], y[:], AF.Exp, scale=-1.0)
            nc.vector.tensor_scalar(out=ey[:], in0=ey[:], scalar1=1.0, scalar2=None,
                                    op0=ALU.add)
            nc.vector.reciprocal(ey[:], ey[:])
            a1 = sp.tile([F // 2, NA], f32, tag="a1")
            nc.vector.tensor_mul(a1[:], y[:], ey[:])
            asum = sp.tile([F // 2, 1], f32, tag="asum")
            nc.vector.reduce_sum(asum[:], a1[:], axis=AX.X)
            en_p = psS.tile([1, 1], f32, tag="nmm")
            nc.tensor.matmul(en_p[:], W["w2"][:], asum[:], start=True, stop=True)
            en = sp.tile([1, 1], f32, tag="en")
            nc.vector.tensor_scalar(out=en[:], in0=en_p[:], scalar1=float(NA * b2),
                                    scalar2=None, op0=ALU.add)
            nc.sync.dma_start(out=energy[:], in_=en[:])

    _split_sync_waits(nc, mybir)
    nc.finalize()
    return nc

